# revision 1
# baseline (speedup 1.0000x reference)
"""Causal multi-head attention (B=4, T=2048, C=1024, H=16, HD=64) on 8 trn2 cores.

Sharding: core i -> batch b = i//2, head-half hh = i%2 (8 heads = 512 dims).
Host sums the two head-half partials per batch.

v2 dataflow (all matmuls bf16 with fp32 PSUM accumulation):
  - All inputs host-prepacked to [128-partition, ...] layouts, one DMA each.
  - v projected directly in [t, d] layout (no PE transposes): per t-tile,
    psum[t,512d] accumulated over 8 c-chunks, copied into vaug[t-tile*8+head,
    0:64]; vaug[..., 64:128] holds ones so the PV matmul also emits the
    softmax denominator on psum partitions 64:128.
  - q/k projected per head-pair into qT/kT [128 d, 2048 t]; the projection
    matmul chunks of pair p+1 are interleaved into pair p's attention loop to
    fill PE gaps while ACT computes exp.
  - Attention in S-transposed layout, software-pipelined by one m-step:
    per tile-pair m, psum sA/sB [128, 1024] hold [j0 | j1] scores per head;
    exp per head on ACT (scale=1/8 fused, bf16 out) ping-pongs against the
    next m's score matmuls; causal mask via DVE multiply with a bf16 mask
    constant; PV(m-1) emitted after scores(m) (held one extra m-step at
    jq boundaries so PE covers the previous chunk's normalize chain);
    diagonal tiles skip their fully-masked leading columns in the score,
    exp, mask and PV ops; normalize = DVE reciprocal+mult straight into
    the persistent attT SBUF tile (bf16, no DRAM spill).
  - Wo phase reads attT from SBUF: y[t-tile, c-half] accumulated over pairs,
    rotating yacc over all 8 PSUM banks; ready groups are interleaved into
    the last pair's attention loop.
  - PE warm-up: dummy matmuls on a memset scratch tile bridge the initial
    DMA window so real matmuls start at full clock.
"""

import sys

import numpy as np

try:
    from concourse import bass, tile, mybir
except ImportError:  # pragma: no cover
    sys.path.insert(0, "/opt/trn_rl_repo")
    from concourse import bass, tile, mybir

from contextlib import ExitStack

from concourse.bass2jax import _bass_exec_p, install_neuronx_cc_hook

F32 = mybir.dt.float32
BF16 = mybir.dt.bfloat16
AF = mybir.ActivationFunctionType
ALU = mybir.AluOpType

B, T, C = 4, 2048, 1024
H, HD = 16, 64
NCORES = 8
HH = 512          # head-dims per core (8 heads)
NPAIR = 4         # head-pairs per core (128 dims each)
NCC = C // 128    # 8 contraction chunks for projections
NTT = T // 128    # 16 t-tiles
NTQ = T // 512    # 4 query chunks

SPLIT_MM_WAITS = True
NORM_VIA_DMA = False

_PROGRAM = None


def _build_masks():
    """bf16 mask constant [128, 2, 4, 512]: case d0//2, col-block 2h+j gets
    mask(d0+j) where mask(d)[p, f] = (128*d + p <= f)."""
    pidx = np.arange(128)[:, None]
    fidx = np.arange(512)[None, :]
    mk = np.zeros((128, 2, 4, 512), np.float32)
    for case, d0 in enumerate((0, 2)):
        for h in range(2):
            for j in range(2):
                mk[:, case, 2 * h + j, :] = (128 * (d0 + j) + pidx <= fidx)
    import ml_dtypes

    return mk.reshape(128, 4096).astype(ml_dtypes.bfloat16).view(np.uint16)


def _build_program():
    nc = bass.Bass("TRN2", target_bir_lowering=False, debug=False)

    xT_d = nc.declare_dram_parameter("xT", [128, NCC, T], BF16, isOutput=False)
    wq_d = nc.declare_dram_parameter("wq", [128, NCC, HH], BF16, isOutput=False)
    wk_d = nc.declare_dram_parameter("wk", [128, NCC, HH], BF16, isOutput=False)
    wv_d = nc.declare_dram_parameter("wv", [128, NCC, HH], BF16, isOutput=False)
    wo_d = nc.declare_dram_parameter("wo", [128, NPAIR, C], BF16, isOutput=False)
    y_d = nc.declare_dram_parameter("y", [T, C], F32, isOutput=True)

    masks_c = nc.inline_tensor(_build_masks(), "masksc")
    import ml_dtypes
    ones_np = np.ones((128, 512), np.float32).astype(ml_dtypes.bfloat16).view(np.uint16)
    ones_c = nc.inline_tensor(ones_np, "onesc")

    xT = xT_d.ap()
    y = y_d.ap()

    with tile.TileContext(nc) as tc, ExitStack() as ctx:
        cst = ctx.enter_context(tc.tile_pool(name="cst", bufs=1))
        qkp = ctx.enter_context(tc.tile_pool(name="qkp", bufs=3))
        esp = ctx.enter_context(tc.tile_pool(name="esp", bufs=3))
        rcp = ctx.enter_context(tc.tile_pool(name="rcp", bufs=2))
        ybp = ctx.enter_context(tc.tile_pool(name="ybp", bufs=8))
        pp = ctx.enter_context(tc.tile_pool(name="pp", bufs=2, space="PSUM"))
        psp = ctx.enter_context(tc.tile_pool(name="psp", bufs=1, space="PSUM"))
        pap = ctx.enter_context(tc.tile_pool(name="pap", bufs=1, space="PSUM"))

        xts = cst.tile([128, NCC, T], BF16, tag="xts")
        wvs = cst.tile([128, NCC, HH], BF16, tag="wvs")
        wqs = cst.tile([128, NCC, HH], BF16, tag="wqs")
        wks = cst.tile([128, NCC, HH], BF16, tag="wks")
        wos = cst.tile([128, NPAIR, C], BF16, tag="wos")
        masks = cst.tile([128, 8, 512], BF16, tag="masks")

        # DMA order = dependency order of the upfront PE work: x chunk 0 and
        # wv feed the first v-projections; masks are not needed until the
        # first diagonal attention tile.
        nc.sync.dma_start(wvs[:, 0:4, :], wv_d.ap()[:, 0:4, :])
        nc.sync.dma_start(xts[:, 0:4, 0:256], xT[:, 0:4, 0:256])
        nc.sync.dma_start(wvs[:, 4:8, :], wv_d.ap()[:, 4:8, :])
        nc.sync.dma_start(xts[:, 4:8, 0:256], xT[:, 4:8, 0:256])
        nc.sync.dma_start(xts[:, :, 256:512], xT[:, :, 256:512])
        nc.sync.dma_start(wqs[:], wq_d.ap()[:])
        nc.sync.dma_start(wks[:], wk_d.ap()[:])
        for t4 in range(1, NTQ):
            nc.sync.dma_start(
                xts[:, :, t4 * 512:(t4 + 1) * 512],
                xT[:, :, t4 * 512:(t4 + 1) * 512],
            )
        nc.sync.dma_start(wos[:], wo_d.ap()[:])
        nc.sync.dma_start(
            masks[:], masks_c.ap()[:].rearrange("p (a c) -> p a c", a=8).bitcast(BF16)
        )

        vaug = cst.tile([128, NTT * 8, 128], BF16, tag="vaug")
        attT = cst.tile([128, NPAIR, T], BF16, tag="attT")

        ones8 = cst.tile([128, 8, 64], BF16, tag="ones8")
        nc.sync.dma_start(ones8[:], ones_c.ap()[:].rearrange("p (a c) -> p a c", a=8).bitcast(BF16))

        # --- filler tasks: PE work interleaved into the ACT-bound attention
        # loop. Each task is (kind, args); emitted one per attention m-step.
        def vproj_task(tt):
            pv = pp.tile([128, 512], F32, tag="pp")
            for cc in range(NCC):
                nc.tensor.matmul(
                    pv[:],
                    xts[:, cc, tt * 128:(tt + 1) * 128],
                    wvs[:, cc, :],
                    start=(cc == 0),
                    stop=(cc == NCC - 1),
                )
            nc.vector.tensor_copy(vaug[:, tt * 8:(tt + 1) * 8, 0:64], pv[:])
            nc.vector.tensor_copy(vaug[:, tt * 8:(tt + 1) * 8, 64:128], ones8[:])

        def alloc_qk():
            qt = qkp.tile([128, T], BF16, tag="qT")
            kt = qkp.tile([128, T], BF16, tag="kT")
            return qt, kt

        def proj_task(p, w_sb, dst, t4):
            acc = pp.tile([128, 512], F32, tag="pp")
            for cc in range(NCC):
                nc.tensor.matmul(
                    acc[:],
                    w_sb[:, cc, p * 128:(p + 1) * 128],
                    xts[:, cc, t4 * 512:(t4 + 1) * 512],
                    start=(cc == 0),
                    stop=(cc == NCC - 1),
                )
            nc.vector.tensor_copy(dst[:, t4 * 512:(t4 + 1) * 512], acc[:])

        def make_proj_tasks(p, qt, kt):
            return [
                (proj_task, (p, w, dst, t4))
                for (w, dst) in ((wqs, qt), (wks, kt))
                for t4 in range(NTQ)
            ]

        def wo_group(ch, tt, yacc, engine):
            for wp_ in range(NPAIR):
                nc.tensor.matmul(
                    yacc,
                    attT[:, wp_, tt * 128:(tt + 1) * 128],
                    wos[:, wp_, ch * 512:(ch + 1) * 512],
                    start=(wp_ == 0),
                    stop=(wp_ == NPAIR - 1),
                )
            yb = ybp.tile([128, 512], F32, tag="yb")
            if engine == "scalar":
                nc.scalar.copy(yb[:], yacc)
            else:
                nc.vector.tensor_copy(yb[:], yacc)
            nc.sync.dma_start(
                y[tt * 128:(tt + 1) * 128, ch * 512:(ch + 1) * 512], yb[:]
            )

        # Wo output groups ordered by earliest readiness (tt ascending);
        # group (ch, tt) needs attT cols of jq = tt//4 from every pair.
        wo_list = [(ch, tt) for tt in range(NTT) for ch in range(2)]

        def emit_pv(paA, paB, p, jt0, e, jq, is_start, is_last):
            for h in range(2):
                pa_ = paA if h == 0 else paB
                hidx = 2 * p + h
                for j in range(2):
                    blk = 2 * h + j
                    off = max(0, (jt0 + j - 4 * jq)) * 128
                    nc.tensor.matmul(
                        pa_[:, off:512],
                        vaug[:, (jt0 + j) * 8 + hidx, :],
                        e[:, blk * 512 + off:(blk + 1) * 512],
                        start=(is_start and j == 0),
                        stop=(is_last and j == 1),
                    )

        # warm the PE during the initial DMA window: dummy matmuls on a
        # memset scratch tile (no data deps, results never read) keep the
        # clock-gate ramp going until the first projection operands land.
        scratch = cst.tile([128, 512], BF16, tag="scratch")
        nc.gpsimd.memset(scratch[:], 0.0)
        for _w in range(16):
            warm = pp.tile([128, 512], F32, tag="pp", name="warm")
            nc.tensor.matmul(
                warm[:], scratch[:, 0:128], scratch[:], start=True, stop=True
            )

        # upfront emission paced to DMA arrival: chunk t4 of x enables both
        # v-proj tiles 4*t4..4*t4+3 and the q/k projection chunk t4.
        qt, kt = alloc_qk()
        p0_proj = make_proj_tasks(0, qt, kt)  # [q-t0..3, k-t0..3]
        for t4 in range(NTQ):
            for tt in range(4 * t4, min(4 * t4 + 4, 12)):
                vproj_task(tt)
            fn, args = p0_proj[t4]
            fn(*args)
            fn, args = p0_proj[NTQ + t4]
            fn(*args)
        # remaining v tiles become filler inside p0's attention; tile 15 is
        # first needed at jq3 (mctr 12), emission at one task per m-step
        # stays well ahead.
        tasks = [(vproj_task, (tt,)) for tt in range(12, NTT)]

        for p in range(NPAIR):
            if p + 1 < NPAIR:
                qt_n, kt_n = alloc_qk()
                tasks.extend(make_proj_tasks(p + 1, qt_n, kt_n))
            else:
                qt_n = kt_n = None
            mctr = 0
            for jq in range(NTQ):
                paA = pap.tile([128, 512], F32, tag="paA")
                paB = pap.tile([128, 512], F32, tag="paB")
                npr = 2 * jq + 2
                pendq = []  # software-pipelined PV args (1 m-step deep; 2 at
                # jq boundaries so PE covers the previous chunk's normalize)
                for m in range(npr):
                    jt0 = 2 * m
                    sA = psp.tile([128, 1024], F32, tag="sA")
                    sB = psp.tile([128, 1024], F32, tag="sB")
                    e = esp.tile([128, 2048], BF16, tag="e")
                    for h, s in ((0, sA), (1, sB)):
                        r0 = h * 64
                        for j in range(2):
                            # columns < 128*d of a diagonal tile are fully
                            # above the causal boundary: skip them
                            off = max(0, (jt0 + j - 4 * jq)) * 128
                            nc.tensor.matmul(
                                s[:, j * 512 + off:(j + 1) * 512],
                                kt[r0:r0 + 64,
                                   (jt0 + j) * 128:(jt0 + j + 1) * 128],
                                qt[r0:r0 + 64, jq * 512 + off:(jq + 1) * 512],
                                start=True,
                                stop=True,
                            )
                        if jt0 < 4 * jq:
                            nc.scalar.activation(
                                e[:, h * 1024:(h + 1) * 1024], s[:],
                                AF.Exp, scale=0.125,
                            )
                        else:
                            # diagonal: exp+mask only the live columns
                            case = (jt0 - 4 * jq) // 2
                            for j in range(2):
                                off = (2 * case + j) * 128
                                blk = 2 * h + j
                                nc.scalar.activation(
                                    e[:, blk * 512 + off:(blk + 1) * 512],
                                    s[:, j * 512 + off:(j + 1) * 512],
                                    AF.Exp, scale=0.125,
                                )
                                nc.vector.tensor_tensor(
                                    e[:, blk * 512 + off:(blk + 1) * 512],
                                    e[:, blk * 512 + off:(blk + 1) * 512],
                                    masks[:, case * 4 + blk, off:512],
                                    ALU.mult,
                                )
                    if not (jq > 0 and m == 1):
                        while pendq:
                            emit_pv(*pendq.pop(0))
                    if tasks:
                        fn, args = tasks.pop(0)
                        fn(*args)
                    elif p == NPAIR - 1 and wo_list and wo_list[0][1] // 4 < jq:
                        # last pair: fill with Wo groups whose attT columns
                        # (jq' = tt//4) are already final for every pair
                        ch_, tt_ = wo_list.pop(0)
                        yacc_t = pp.tile([128, 512], F32, tag="pp", name="yacc_t")
                        wo_group(ch_, tt_, yacc_t[:], "vector")
                    mctr += 1
                    pendq.append(
                        (paA, paB, p, jt0, e, jq, m == 0, m == npr - 1)
                    )
                for args in pendq:
                    emit_pv(*args)
                for h, pa_ in ((0, paA), (1, paB)):
                    rc = rcp.tile([64, 512], F32, tag="rc")
                    nc.vector.reciprocal(rc[:], pa_[64:128, :])
                    if h == 0 or not NORM_VIA_DMA:
                        nc.vector.tensor_tensor(
                            attT[h * 64:(h + 1) * 64, p, jq * 512:(jq + 1) * 512],
                            pa_[0:64, :],
                            rc[:],
                            ALU.mult,
                        )
                    else:
                        st = rcp.tile([64, 512], BF16, tag="st")
                        nc.vector.tensor_tensor(
                            st[:], pa_[0:64, :], rc[:], ALU.mult
                        )
                        nc.sync.dma_start(
                            attT[64:128, p, jq * 512:(jq + 1) * 512], st[:]
                        )
            # flush any leftover tasks so pair p+1's projections are always
            # emitted (and hence ordered) before its attention loop
            while tasks:
                fn, args = tasks.pop(0)
                fn(*args)
            qt, kt = qt_n, kt_n

        # --- Wo phase: y[tt, ch] = sum_p attT_p[:, tt].T @ wo_p[:, ch]
        # yacc rotates over all 8 PSUM banks (reusing the attention pools'
        # banks) so matmul groups run ahead of the evacuation copies, which
        # alternate between ACT and DVE.
        sA_w = psp.tile([128, 1024], F32, tag="sA")
        sB_w = psp.tile([128, 1024], F32, tag="sB")
        paA_w = pap.tile([128, 512], F32, tag="paA")
        paB_w = pap.tile([128, 512], F32, tag="paB")
        fixed_accs = [
            sA_w[:, 0:512], sA_w[:, 512:1024],
            sB_w[:, 0:512], sB_w[:, 512:1024],
            paA_w[:], paB_w[:],
        ]
        idx = 0
        while wo_list:
            ch_, tt_ = wo_list.pop(0)
            k8 = idx % 8
            if k8 < 2:
                yacc_t = pp.tile([128, 512], F32, tag="pp", name="yacc_t")
                yacc = yacc_t[:]
            else:
                yacc = fixed_accs[k8 - 2]
            idx += 1
            wo_group(ch_, tt_, yacc, "scalar" if idx % 2 == 0 else "vector")

    if SPLIT_MM_WAITS:
        _split_matmul_waits(nc)
    return nc


def _split_matmul_waits(nc):
    """walrus's fused-LDW matmul lowering can't carry multiple sync waits
    (S3_LW setupSyncWait assert). Move every matmul's waits onto a preceding
    same-engine NoOp, which lowers with full sync support."""
    f = nc.m.functions[0]
    k = 0
    for bb in f.blocks:
        insts = bb.instructions
        out = []
        for i in insts:
            waits = list(i.sync_info.on_wait) if i.sync_info is not None else []
            keep = 0 if type(i).__name__ == "InstMatmult" else 1
            if len(waits) > keep:
                moved, kept = waits[: len(waits) - keep], waits[len(waits) - keep:]
                for w in moved:
                    n = mybir.InstNoOp(name=f"I-mmwait{k}")
                    k += 1
                    n.engine = i.engine
                    n.sync_info = mybir.SyncInfo(on_wait=[w], on_update=[])
                    nc.register_instruction(n)
                    out.append(n)
                i.sync_info = mybir.SyncInfo(
                    on_wait=kept, on_update=list(i.sync_info.on_update)
                )
            out.append(i)
        if k:
            bb.instructions = out


def _get_program():
    global _PROGRAM
    if _PROGRAM is None:
        _PROGRAM = _build_program()
    return _PROGRAM


_RUNNER = None


def _get_runner():
    """Compile the SPMD program into a cached sharded jit callable."""
    global _RUNNER
    if _RUNNER is not None:
        return _RUNNER
    import jax
    from jax.experimental.shard_map import shard_map
    from jax.sharding import Mesh, PartitionSpec

    nc = _get_program()
    install_neuronx_cc_hook()

    partition_name = (
        nc.partition_id_tensor.name if nc.partition_id_tensor else None
    )
    in_names, out_names, out_avals = [], [], []
    for alloc in nc.m.functions[0].allocations:
        if not isinstance(alloc, mybir.MemoryLocationSet):
            continue
        name = alloc.memorylocations[0].name
        if alloc.kind == "ExternalInput":
            if name != partition_name:
                in_names.append(name)
        elif alloc.kind == "ExternalOutput":
            out_names.append(name)
            out_avals.append(
                jax.core.ShapedArray(tuple(alloc.tensor_shape), mybir.dt.np(alloc.dtype))
            )
    n_params = len(in_names)
    zero_outs = [np.zeros(a.shape, a.dtype) for a in out_avals]
    all_in_names = list(in_names) + list(out_names)
    if partition_name is not None:
        all_in_names.append(partition_name)
    all_in_names = tuple(all_in_names)

    def _body(*args):
        operands = list(args)
        if partition_name is not None:
            from concourse.bass2jax import partition_id_tensor

            operands.append(partition_id_tensor())
        outs = _bass_exec_p.bind(
            *operands,
            out_avals=tuple(out_avals),
            in_names=all_in_names,
            out_names=tuple(out_names),
            lowering_input_output_aliases=(),
            sim_require_finite=True,
            sim_require_nnan=True,
            nc=nc,
        )
        return tuple(outs)

    devices = jax.devices()[:NCORES]
    assert len(devices) == NCORES, devices
    mesh = Mesh(np.asarray(devices), ("core",))
    n_all = n_params + len(out_names)
    sharded = jax.jit(
        shard_map(
            _body,
            mesh=mesh,
            in_specs=(PartitionSpec("core"),) * n_all,
            out_specs=(PartitionSpec("core"),) * len(out_names),
            check_rep=False,
        ),
        keep_unused=True,
    )
    _RUNNER = dict(
        sharded=sharded,
        in_names=in_names,
        out_names=out_names,
        out_avals=out_avals,
        zero_outs=zero_outs,
        mesh=mesh,
    )
    return _RUNNER


def _run(in_maps):
    r = _get_runner()
    concat_in = [
        np.concatenate([np.asarray(m[name]) for m in in_maps], axis=0)
        for name in r["in_names"]
    ]
    concat_zeros = [
        np.zeros((NCORES * z.shape[0], *z.shape[1:]), z.dtype) for z in r["zero_outs"]
    ]
    out_arrs = r["sharded"](*concat_in, *concat_zeros)
    return [
        {
            name: np.asarray(out_arrs[i]).reshape(NCORES, *r["out_avals"][i].shape)[c]
            for i, name in enumerate(r["out_names"])
        }
        for c in range(NCORES)
    ]


def timed_run(in_maps, iters=10):
    """Execute with inputs pre-staged on device; return per-iteration seconds."""
    import time
    import jax

    r = _get_runner()
    concat_in = [
        np.concatenate([np.asarray(m[name]) for m in in_maps], axis=0)
        for name in r["in_names"]
    ]
    concat_zeros = [
        np.zeros((NCORES * z.shape[0], *z.shape[1:]), z.dtype) for z in r["zero_outs"]
    ]
    from jax.sharding import NamedSharding, PartitionSpec

    sh = NamedSharding(r["mesh"], PartitionSpec("core"))
    args = [jax.device_put(a, sh) for a in concat_in + concat_zeros]
    out = r["sharded"](*args)  # warmup + compile
    jax.block_until_ready(out)
    times = []
    for _ in range(iters):
        t0 = time.perf_counter()
        out = r["sharded"](*args)
        jax.block_until_ready(out)
        times.append(time.perf_counter() - t0)
    return times


def make_in_maps(x, Wq, Wk, Wv, Wo):
    import ml_dtypes

    BF = ml_dtypes.bfloat16

    def pack_cpart(a, nblk):
        # [nblk*128, F] -> [128, nblk, F]
        f = a.shape[1]
        return np.ascontiguousarray(
            a.reshape(nblk, 128, f).transpose(1, 0, 2)
        ).astype(BF)

    x = np.asarray(x, dtype=np.float32)
    xTs = [pack_cpart(np.ascontiguousarray(x[b].T), NCC) for b in range(B)]
    Wq = np.asarray(Wq, dtype=np.float32)
    Wk = np.asarray(Wk, dtype=np.float32)
    Wv = np.asarray(Wv, dtype=np.float32)
    Wo = np.asarray(Wo, dtype=np.float32)
    in_maps = []
    for core in range(NCORES):
        b, hh = core // 2, core % 2
        sl = slice(hh * HH, (hh + 1) * HH)
        in_maps.append({
            "xT": xTs[b],
            "wq": pack_cpart(np.ascontiguousarray(Wq[sl, :].T), NCC),
            "wk": pack_cpart(np.ascontiguousarray(Wk[sl, :].T), NCC),
            "wv": pack_cpart(np.ascontiguousarray(Wv[sl, :].T), NCC),
            "wo": pack_cpart(np.ascontiguousarray(Wo[:, sl].T), NPAIR),
        })
    return in_maps


def kernel(x, Wq, Wk, Wv, Wo):
    in_maps = make_in_maps(x, Wq, Wk, Wv, Wo)
    results = _run(in_maps)
    out = np.empty((B, T, C), dtype=np.float32)
    for b in range(B):
        out[b] = results[2 * b]["y"] + results[2 * b + 1]["y"]
    return out

